# revision 1
# baseline (speedup 1.0000x reference)
"""BinEmbedding kernel for Trainium2 (8 NeuronCores, data-parallel).

out[b, l, :] = emb_table[tok(x[b, l])]
  tok = 0 for NaN x, else clamp(searchsorted(bins, x, 'right') - 1, 0) + 1
      = [x >= -3e38] + sum_{j=1..255} [x >= bins[j]]   (exact fp32 is_ge; NaN
        compares false everywhere -> 0)

Per core: x slab [128, 1024] f32, element e = p*1024 + c at x_sb[p, c].
VectorE: 256 fused is_ge+add passes (bin thresholds baked as immediates),
cast to int16 tokens. Gather: SWDGE dma_gather of 256-B table rows, 1024
indices per call (128 calls) -- the call's indices are a 16-row band slice
of tok16, reshuffled within-partition by DVE into the HW's wrapped idx
order and replicated to partitions [0:32) (rx+tx Q7 cores) by one DMA per
col-block. Output lands so each dst partition holds 8 consecutive out rows
per call; stores use a split-partition AP (q outer, u inner).

Call (k, b, s): band k (x rows 16k..16k+16), col window C0 = b*256 + s*64.
  gather slot i = 128*s2 + 16*u + q  ->  dst[16u+q, s2]
  element e(q, u, s2) = (16k+q)*1024 + C0 + 8u + s2
  idx wrap: idx[q, 8*s2+u] = tok16[16k+q, C0 + 8u + s2]
"""

import sys

sys.path.insert(0, "/opt/trn_rl_repo")

import numpy as np

import concourse.bacc as bacc
import concourse.bass as bass
import concourse.mybir as mybir
from concourse.bass_utils import run_bass_kernel_spmd
from concourse.library_config import mlp

B, L = 16, 65536
NUM_BINS = 256
H = 64
P = 128
NCORES = 8

COLS = 1024
BLOCK_COLS = 256          # DVE compute block
SUB_COLS = 64             # cols per gather call
NI = 16 * SUB_COLS        # 1024 idxs per gather call (HW-safe limit)
NBUF = 16                 # dst ring depth (hides DMA latency)


def build_nc(bins: np.ndarray, cols: int = COLS):
    assert bins.shape == (NUM_BINS,) and bins.dtype == np.float32
    nblocks = cols // BLOCK_COLS
    subs_per_block = BLOCK_COLS // SUB_COLS        # 4
    calls_per_block = 8 * subs_per_block           # 32 (8 bands)
    ncalls = nblocks * calls_per_block

    thr = [-3.0e38] + [float(v) for v in bins[1:]]

    nc = bacc.Bacc("TRN2", target_bir_lowering=False, debug=False,
                   detect_race_conditions=False)
    x_d = nc.dram_tensor("x", [P, cols], mybir.dt.float32, kind="ExternalInput")
    emb_d = nc.dram_tensor(
        "emb", [NUM_BINS + 1, H], mybir.dt.float32, kind="ExternalInput"
    )
    out_d = nc.dram_tensor(
        "out", [P, cols * H], mybir.dt.float32, kind="ExternalOutput"
    )

    with (
        nc.sbuf_tensor("x_sb", [P, cols], mybir.dt.float32) as x_sb,
        nc.sbuf_tensor("acc", [P, cols], mybir.dt.float32) as acc,
        nc.sbuf_tensor("tok", [P, cols], mybir.dt.int16) as tok,
        nc.sbuf_tensor("strips", [P, cols], mybir.dt.int16) as strips,
        nc.sbuf_tensor("idxb", [P, (cols // SUB_COLS) * 8 * (NI // 16)], mybir.dt.int16) as idxb,
        nc.sbuf_tensor("dst", [P, NBUF, NI // P, H], mybir.dt.float32) as dst,
        nc.semaphore("sem_x") as sem_x,
        nc.semaphore("sem_strip") as sem_strip,
        nc.semaphore("sem_rep") as sem_rep,
        nc.semaphore("sem_v") as sem_v,
        nc.semaphore("sg0") as sg0,
        nc.semaphore("sg1") as sg1,
        nc.semaphore("sg2") as sg2,
        nc.semaphore("sg3") as sg3,
        nc.semaphore("ss0") as ss0,
        nc.semaphore("ss1") as ss1,
        nc.semaphore("ss2") as ss2,
        nc.semaphore("ss3") as ss3,
        nc.Block() as block,
    ):
        sem_gd = [sg0, sg1, sg2, sg3]
        sem_st = [ss0, ss1, ss2, ss3]

        # call index -> (b, k, s); processed in order b, then k-major, s inner
        def call_info(i):
            b, r = divmod(i, calls_per_block)
            k, s = divmod(r, subs_per_block)
            return b, k, s

        @block.vector
        def _(vector):
            # Intra-DVE RAW hazards are safe on HW (the per-op DRAIN is the
            # output-dependency barrier); only cross-engine edges get sems.
            vector.memset(strips[:, :], 0).then_inc(sem_strip, 1)
            vector.memset(idxb[:, :], 0).then_inc(sem_strip, 1)
            vector.memset(dst[:, :, :, :], 0)
            vector.wait_ge(sem_x, 16)
            for b in range(nblocks):
                lo, hi = b * BLOCK_COLS, (b + 1) * BLOCK_COLS
                xs = x_sb[:, lo:hi]
                ac = acc[:, lo:hi]
                vector.tensor_scalar(ac, xs, thr[0], None, mybir.AluOpType.is_ge)
                for j in range(1, NUM_BINS):
                    vector.scalar_tensor_tensor(
                        ac, xs, thr[j], ac,
                        mybir.AluOpType.is_ge, mybir.AluOpType.add,
                    )
                vector.tensor_copy(tok[:, lo:hi], ac)
                # strip shuffle: strips[16k+q, C0+8*s2+u] = tok[16k+q, C0+8u+s2]
                # one op per 32-partition band pair (legal start partitions)
                for m in range(4):
                    pr = slice(32 * m, 32 * m + 32)
                    o = strips[pr, lo:hi].rearrange(
                        "p (s a c) -> p s a c", a=8, c=8
                    )
                    i_ = tok[pr, lo:hi].rearrange(
                        "p (s c a) -> p s a c", c=8, a=8
                    )
                    vector.tensor_copy(o, i_).then_inc(sem_strip, 1)

        @block.scalar
        def _(scalar):
            # replicate strip bands into the wrapped idx tile, 2 copies
            # (partitions 0:16 rx and 16:32 tx), one DMA per col-block:
            # idxb[cp*16+q, b*2048 + (k*4+s)*64 + j] = strips[16k+q, b*256+s*64+j]
            for b in range(nblocks):
                scalar.wait_ge(sem_strip, 2 + 4 * (b + 1))
                if b >= 1:
                    # previous block's replication fully complete -> sem_rep
                    # milestones stay unambiguous (one block in flight)
                    scalar.wait_ge(sem_rep, 256 * b)
                for cp in range(2):
                    for k in range(8):
                        src_ap = strips[16 * k : 16 * k + 16,
                                        b * BLOCK_COLS : (b + 1) * BLOCK_COLS]
                        base = (b * calls_per_block + k * subs_per_block) * (NI // 16)
                        dst_ap = idxb[cp * 16 : cp * 16 + 16,
                                      base : base + BLOCK_COLS]
                        scalar.dma_start(dst_ap, src_ap).then_inc(sem_rep, 16)

        @block.gpsimd
        def _(gpsimd):
            gpsimd.load_library(mlp)
            for i in range(ncalls):
                b, k, s = call_info(i)
                gpsimd.wait_ge(sem_rep, 256 * (b + 1))
                if i >= NBUF:
                    j = i - NBUF
                    gpsimd.wait_ge(sem_st[j % 4], 16 * (j // 4 + 1))
                gpsimd.dma_gather(
                    dst[:, i % NBUF, :, :],
                    emb_d[:, :],
                    idxb[:, i * (NI // 16) : (i + 1) * (NI // 16)],
                    NI,
                    NI,
                    H,
                ).then_inc(sem_gd[i % 4], 16)

        @block.sync
        def _(sync):
            sync.dma_start(x_sb[:, :], x_d[:, :]).then_inc(sem_x, 16)
            for i in range(ncalls):
                b, k, s = call_info(i)
                c0 = b * BLOCK_COLS + s * SUB_COLS
                sync.wait_ge(sem_gd[i % 4], 16 * (i // 4 + 1))
                # dst[P = 16u+q, s2, h] -> out row (16k+q)*1024 + c0 + 8u + s2
                # SBUF side: natural partition order P (q fastest);
                # DRAM side: dims (u outer, q inner) to match.
                out_ap = bass.AP(
                    out_d,
                    (16 * k) * (cols * H) + c0 * H,
                    [
                        [8 * H, 8],          # u
                        [cols * H, 16],      # q (out rows)
                        [1, 8 * H],          # s2*H + h contiguous
                    ],
                )
                src_ap = dst[:, i % NBUF, :, :].rearrange("p a h -> p (a h)")
                sync.dma_start(out_ap, src_ap).then_inc(sem_st[i % 4], 16)

    nc.compile()
    return nc


_CACHE: dict = {}


def _get_nc(bins: np.ndarray):
    key = bins.tobytes()
    if key not in _CACHE:
        _CACHE[key] = build_nc(bins)
    return _CACHE[key]


def kernel(x: np.ndarray, bins: np.ndarray, emb_table: np.ndarray) -> np.ndarray:
    x = np.asarray(x, dtype=np.float32)
    bins = np.asarray(bins, dtype=np.float32)
    emb_table = np.asarray(emb_table, dtype=np.float32)
    assert x.shape == (B, L) and emb_table.shape == (NUM_BINS + 1, H)

    nc = _get_nc(bins)
    rows_per_core = B // NCORES
    in_maps = [
        {
            "x": x[i * rows_per_core : (i + 1) * rows_per_core].reshape(P, -1).copy(),
            "emb": emb_table,
        }
        for i in range(NCORES)
    ]
    res = run_bass_kernel_spmd(nc, in_maps, core_ids=list(range(NCORES)))
    outs = [
        res.results[i]["out"].reshape(rows_per_core, L, H) for i in range(NCORES)
    ]
    return np.concatenate(outs, axis=0)


if __name__ == "__main__":
    import concourse.bass_interp as bass_interp

    rng = np.random.default_rng(0)
    n = P * COLS
    bins = np.sort(rng.standard_normal(NUM_BINS).astype(np.float32) * 1.5)
    emb = rng.standard_normal((NUM_BINS + 1, H)).astype(np.float32)
    xs = rng.standard_normal(n).astype(np.float32)
    xs[rng.random(n) < 0.1] = np.nan

    nc = build_nc(bins)
    sim = bass_interp.CoreSim(nc, require_nnan=False, require_finite=False)
    sim.tensor("x")[:] = xs.reshape(P, COLS)
    sim.tensor("emb")[:] = emb
    sim.simulate()
    got = np.asarray(sim.tensor("out")).reshape(n, H)

    nans = np.isnan(xs)
    xc = np.where(nans, 0.0, xs)
    idx = np.maximum(np.searchsorted(bins, xc, side="right") - 1, 0)
    tok_ref = np.where(nans, 0, idx + 1)
    want = emb[tok_ref]
    err = np.abs(got - want).max()
    print("sim absmax err:", err)
    print("sim time estimate:", sim.time, "ns")
    assert err == 0.0, err
    print("SIM OK")

